# revision 14
# baseline (speedup 1.0000x reference)
"""Causal self-attention (B=4, T=2048, C=1024, 16 heads) on 8 trn2 NeuronCores.

Sharding: core (b, hg) handles batch b (4) x head-group hg (2 groups of 8 heads).
Each core computes QKV projection for its batch restricted to its 8 heads,
flash-style causal attention, and the output projection restricted to its
heads' rows of w_proj -> a partial [T, C] output. Host sums the two partials
per batch (tensor-parallel unshard) and concatenates batches.

Key layout choices (all bf16 matmul inputs, fp32 PSUM accumulation):
  - x is fed pre-transposed per batch: xT [C, T], so Q^T/K^T [d, t] come
    straight out of the QKV matmuls (lhsT = w slice, rhs = xT).
  - Scores are computed TRANSPOSED: S^T[tk, q] = (K^T)^T-style matmul with
    lhsT = K^T chunk, rhs = Q^T chunk. Softmax exp runs on ScalarE from PSUM.
  - V carries an appended ones-column, so the A@V matmul (lhsT=[V|1], rhs=P^T,
    N=512 column-dense streams that keep the PE's HAM activity monitor at full
    clock) yields y~^T = [64 weighted-V | row-sum] x q in one pass. The [65,512]
    result is PE-transposed in 128-q blocks so the softmax denominator becomes
    per-partition -> cheap DVE reciprocal + tensor_scalar multiply.
  - y [t, ch] is PE-transposed to y^T [ch, t] for the output projection.
  - Causal masking: block-skipping, matmul column ranges narrowed to the valid
    q-range on diagonal slots, exp written only to the valid range of a
    dedicated diag buffer (whose sub-diagonal zeros are written once), and one
    [128,128] staircase mask multiplied into the true diagonal blocks.
  - Next pair's QKV projection matmuls are interleaved into the attention
    c-loop as PE filler while ScalarE works through the exps; pair 3
    interleaves the transpose/output-projection tail the same way.
"""

import numpy as np
import ml_dtypes

B, T, C, H, D = 4, 2048, 1024, 16, 64
P = 128
TC = T // P          # 16 t-chunks of 128
KC = C // P          # 8 contraction chunks of 128
NPAIR = 4            # head pairs per core (8 local heads)
SCALE = 0.125        # 1/sqrt(64)

_CACHE = {}
LAST_RESULT = None   # BassKernelResults of the most recent run (for test.py)

BF16 = ml_dtypes.bfloat16


def _build_program():
    import concourse.tile as tile
    import concourse.mybir as mybir
    from concourse import bacc

    dt = mybir.dt
    AF = mybir.ActivationFunctionType
    ALU = mybir.AluOpType

    nc = bacc.Bacc("TRN2", target_bir_lowering=False, debug=False, num_devices=8)

    # ---- DRAM I/O ----
    xT_d = nc.dram_tensor("xT", [C, T], dt.bfloat16, kind="ExternalInput").ap()
    wqk_d = nc.dram_tensor("wqk", [C, 1024], dt.bfloat16, kind="ExternalInput").ap()
    wv_d = nc.dram_tensor("wv", [C, 512], dt.bfloat16, kind="ExternalInput").ap()
    wproj_d = nc.dram_tensor("wproj", [512, C], dt.bfloat16, kind="ExternalInput").ap()
    bqk_d = nc.dram_tensor("bqk", [P, 8], dt.float32, kind="ExternalInput").ap()
    bv_d = nc.dram_tensor("bv", [P, 512], dt.float32, kind="ExternalInput").ap()
    bproj_d = nc.dram_tensor("bproj", [P, C], dt.float32, kind="ExternalInput").ap()
    dmask_d = nc.dram_tensor("dmask", [P, P], dt.bfloat16, kind="ExternalInput").ap()
    ident_d = nc.dram_tensor("ident", [P, P], dt.bfloat16, kind="ExternalInput").ap()
    ident32_d = nc.dram_tensor("ident32", [P, P], dt.float32, kind="ExternalInput").ap()
    out_d = nc.dram_tensor("out", [T, C], dt.bfloat16, kind="ExternalOutput").ap()

    with tile.TileContext(nc) as tc:
        with (
            tc.tile_pool(name="const", bufs=1) as cp,
            tc.tile_pool(name="outp", bufs=4) as op_pool,
            tc.tile_pool(name="small", bufs=8) as sp,
            tc.tile_pool(name="ytmp", bufs=3) as yt_pool,
            tc.tile_pool(name="psqk", bufs=2, space="PSUM") as psqk_pool,
            tc.tile_pool(name="psmm", bufs=2, space="PSUM") as psmm_pool,
            tc.tile_pool(name="pstr", bufs=2, space="PSUM") as pstr_pool,
        ):
            # ---- static SBUF tensors ----
            xT_s = cp.tile([P, KC, T], dt.bfloat16, name="xT_s")
            wqk_s = cp.tile([P, KC, 1024], dt.bfloat16, name="wqk_s")
            wv_s = cp.tile([P, KC, 512], dt.bfloat16, name="wv_s")
            wproj_s = cp.tile([P, 4, C], dt.bfloat16, name="wproj_s")
            bqk_s = cp.tile([P, 8], dt.float32, name="bqk_s")
            bv_s = cp.tile([P, 512], dt.float32, name="bv_s")
            bproj_s = cp.tile([P, C], dt.float32, name="bproj_s")
            dmask_s = cp.tile([P, P], dt.bfloat16, name="dmask_s")
            ident_s = cp.tile([P, P], dt.bfloat16, name="ident_s")
            ident32_s = cp.tile([P, P], dt.float32, name="ident32_s")
            qt_s = cp.tile([P, NPAIR, T], dt.bfloat16, name="qt_s")  # later reused as y^T
            kt_s = cp.tile([P, NPAIR, T], dt.bfloat16, name="kt_s")
            v_s = cp.tile([P, TC, 8, 66], dt.bfloat16, name="v_s")   # [t, tc, head, V|1]
            y_s = cp.tile([P, TC, 8, D], dt.bfloat16, name="y_s")    # y natural [q, head, d]
            pt_s = cp.tile([P, 2, 6, 1024], dt.bfloat16, name="pt_s")   # exp(S^T), off-diag groups
            ptd_s = cp.tile([P, 2, 2, 1024], dt.bfloat16, name="ptd_s")  # diag groups (zeros persist)

            # ---- input DMAs (weights for the first compute first) ----
            xT_src = xT_d.rearrange("(o p) t -> p o t", p=P)
            wv_src = wv_d.rearrange("(o p) m -> p o m", p=P)
            wqk_src = wqk_d.rearrange("(o p) m -> p o m", p=P)
            nc.sync.dma_start(wv_s[:, 0:4, :], wv_src[:, 0:4, :])
            nc.sync.dma_start(xT_s[:, :, 0:256], xT_src[:, :, 0:256])
            nc.sync.dma_start(wv_s[:, 4:8, :], wv_src[:, 4:8, :])
            nc.sync.dma_start(bv_s[:], bv_d)
            nc.sync.dma_start(xT_s[:, :, 256:512], xT_src[:, :, 256:512])
            nc.sync.dma_start(wqk_s[:, 0:4, :], wqk_src[:, 0:4, :])
            nc.sync.dma_start(xT_s[:, :, 512:768], xT_src[:, :, 512:768])
            nc.sync.dma_start(wqk_s[:, 4:8, :], wqk_src[:, 4:8, :])
            nc.sync.dma_start(bqk_s[:], bqk_d)
            for q8 in range(3, 8):
                nc.sync.dma_start(xT_s[:, :, 256 * q8:256 * (q8 + 1)],
                                  xT_src[:, :, 256 * q8:256 * (q8 + 1)])
            nc.sync.dma_start(dmask_s[:], dmask_d)
            nc.sync.dma_start(ident_s[:], ident_d)
            nc.sync.dma_start(ident32_s[:], ident32_d)
            nc.sync.dma_start(wproj_s[:], wproj_d.rearrange("(o p) m -> p o m", p=P))
            nc.sync.dma_start(bproj_s[:], bproj_d)

            # ones column of V~; zero the diag P^T buffer once (sub-diagonal
            # regions are never written by the partial exps, so zeros persist)
            nc.vector.memset(v_s[:, :, :, 64:65], 1.0)
            nc.vector.memset(ptd_s[:], 0.0)

            # ---- V projection: v[t, ch] for all 8 heads (512 cols) ----
            for tcx in range(TC):
                psv = psmm_pool.tile([P, 512], dt.float32, name="psv", tag="mm")
                for k in range(KC):
                    nc.tensor.matmul(psv[:, :],
                                     xT_s[:, k, P * tcx:P * (tcx + 1)],
                                     wv_s[:, k, :],
                                     start=(k == 0), stop=(k == KC - 1))
                nc.vector.tensor_add(
                    out=v_s[:, tcx, :, 0:64],
                    in0=psv[:, :].rearrange("a (h d) -> a h d", h=8),
                    in1=bv_s[:, :].rearrange("a (h d) -> a h d", h=8),
                )

            # ---- helper emitters ----
            def qkproj_chunk(m, t4):
                """One [128 out-ch, 512 t] tile of the Q^T/K^T projection."""
                dst = qt_s if m < 4 else kt_s
                psq = psmm_pool.tile([P, 512], dt.float32, name="psq", tag="mm")
                for k in range(KC):
                    nc.tensor.matmul(psq[:, :],
                                     wqk_s[:, k, P * m:P * (m + 1)],
                                     xT_s[:, k, 512 * t4:512 * (t4 + 1)],
                                     start=(k == 0), stop=(k == KC - 1))
                nc.vector.tensor_scalar(
                    out=dst[:, m % 4, 512 * t4:512 * (t4 + 1)],
                    in0=psq[:, :], scalar1=bqk_s[:, m:m + 1], scalar2=None,
                    op0=ALU.add)

            def transpose_chunk(tcx, cc):
                pst = pstr_pool.tile([P, P], dt.bfloat16, name="pst", tag="tr")
                nc.tensor.transpose(pst[:, :], y_s[:, tcx, 2 * cc:2 * cc + 2, :],
                                    ident_s[:, :])
                nc.scalar.copy(out=qt_s[:, cc, P * tcx:P * (tcx + 1)],
                               in_=pst[:, :])

            def proj_chunk(tcx):
                for co in range(2):
                    psp = psmm_pool.tile([P, 512], dt.float32, name="psp", tag="mm")
                    for cc in range(4):
                        nc.tensor.matmul(psp[:, :],
                                         qt_s[:, cc, P * tcx:P * (tcx + 1)],
                                         wproj_s[:, cc, 512 * co:512 * (co + 1)],
                                         start=(cc == 0), stop=(cc == 3))
                    ot = op_pool.tile([P, 512], dt.bfloat16, name="ot", tag="ot")
                    nc.vector.tensor_add(out=ot[:, :], in0=psp[:, :],
                                         in1=bproj_s[:, 512 * co:512 * (co + 1)])
                    nc.sync.dma_start(
                        out_d[P * tcx:P * (tcx + 1), 512 * co:512 * (co + 1)], ot[:, :])

            # ---- pair 0 projection upfront; later pairs interleave ----
            for m in (0, 4):
                for t4 in range(4):
                    qkproj_chunk(m, t4)

            for pair in range(NPAIR):
                nxt = ([(m, t4) for m in (pair + 1, 5 + pair) for t4 in range(4)]
                       if pair < NPAIR - 1 else [])
                c_order = range(4) if pair < NPAIR - 1 else (3, 2, 1, 0)
                for ci, c in enumerate(c_order):    # q chunk of 512
                    ngroups = 2 * c + 2     # tk-chunk pairs: j = 2g, 2g+1
                    for g in range(ngroups):
                        diag = g >= 2 * c
                        # both heads' score tiles; MMs interleaved h0/h1 so
                        # row-tiled (K=64) pairs overlap in the PE array
                        psS = [psqk_pool.tile([P, 1024], dt.float32, name="psS",
                                              tag="psqk") for _ in (0, 1)]
                        for s in (0, 1):
                            j = 2 * g + s
                            q0 = P * (j - 4 * c) if diag else 0  # skip masked cols
                            for hh in (0, 1):
                                base = 64 * hh
                                nc.tensor.matmul(
                                    psS[hh][:, 512 * s + q0:512 * (s + 1)],
                                    kt_s[base:base + 64, pair, P * j:P * (j + 1)],
                                    qt_s[base:base + 64, pair,
                                         512 * c + q0:512 * (c + 1)],
                                    start=True, stop=True)
                        # exp( S^T * scale ), fp32 psum -> bf16 sbuf
                        for hh in (0, 1):
                            if not diag:
                                nc.scalar.activation(pt_s[:, hh, g, :], psS[hh][:, :],
                                                     AF.Exp, scale=SCALE)
                            else:
                                gd = g - 2 * c
                                for s in (0, 1):
                                    r = 2 * gd + s
                                    q0 = 512 * s + P * r
                                    nc.scalar.activation(
                                        ptd_s[:, hh, gd, q0:512 * (s + 1)],
                                        psS[hh][:, q0:512 * (s + 1)],
                                        AF.Exp, scale=SCALE)
                                    # staircase mask on the true diagonal block
                                    nc.vector.tensor_tensor(
                                        out=ptd_s[:, hh, gd, q0:q0 + P],
                                        in0=ptd_s[:, hh, gd, q0:q0 + P],
                                        in1=dmask_s[:, :], op=ALU.mult)

                    # PE filler while ScalarE works through the exps:
                    # next pair's Q^T/K^T projection, 2 chunks per c
                    for (m, t4) in nxt[2 * ci:2 * ci + 2]:
                        qkproj_chunk(m, t4)

                    # [V | 1]^T @ P^T per head: y~^T [65, 512] column-dense
                    # (keeps the PE MAC-active so HAM stays at full clock),
                    # then PE-transpose 128-col blocks to get the softmax
                    # denominator per-partition for the division
                    for hh in (0, 1):
                        h = 2 * pair + hh
                        nj = 4 * c + 4
                        psyt = psmm_pool.tile([P, 512], dt.float32, name="psyt",
                                              tag="mm")
                        for j in range(nj):
                            g, s = j // 2, j % 2
                            if g < 2 * c:
                                rhs = pt_s[:, hh, g, 512 * s:512 * (s + 1)]
                                out = psyt[0:65, :]
                            else:
                                # diagonal slot: only columns q >= 128r are live
                                r = j - 4 * c
                                q0 = P * r
                                rhs = ptd_s[:, hh, g - 2 * c,
                                            512 * s + q0:512 * (s + 1)]
                                out = psyt[0:65, q0:]
                            nc.tensor.matmul(
                                out, v_s[:, j, h, 0:65], rhs,
                                start=(j == 0), stop=(j == nj - 1))
                        ytmp = yt_pool.tile([P, 512], dt.float32, name="ytmp",
                                            tag="ytmp")
                        nc.vector.tensor_copy(out=ytmp[0:65, :], in_=psyt[0:65, :])
                        for qi_loc in range(4):
                            qi = 4 * c + qi_loc
                            ptr = pstr_pool.tile([P, P], dt.float32, name="ptr",
                                                 tag="tr")
                            nc.tensor.transpose(
                                ptr[:, 0:65],
                                ytmp[0:65, P * qi_loc:P * (qi_loc + 1)],
                                ident32_s[0:65, 0:65])
                            linv = sp.tile([P, 1], dt.float32, name="linv", tag="linv")
                            nc.vector.reciprocal(linv[:, :], ptr[:, 64:65])
                            nc.vector.tensor_scalar(
                                out=y_s[:, qi, h, :], in0=ptr[:, 0:64],
                                scalar1=linv[:, :], scalar2=None, op0=ALU.mult)

                    if pair == NPAIR - 1:
                        # tail pipelined into pair 3: transpose y -> y^T
                        # (reusing qt_s) and run the output projection for the
                        # t-chunks whose y rows just completed
                        for qi_loc in range(4):
                            tcx = 4 * c + qi_loc
                            for cc in range(4):
                                transpose_chunk(tcx, cc)
                        for qi_loc in range(4):
                            proj_chunk(4 * c + qi_loc)

    nc.compile()
    return nc


def _prep_inputs(x, w_attn, b_attn, w_proj, b_proj):
    """Host-side shard prep: per-core input dicts (core ci = b*2 + hg)."""
    x = np.asarray(x, dtype=np.float32)
    w_attn = np.asarray(w_attn, dtype=np.float32)
    b_attn = np.asarray(b_attn, dtype=np.float32)
    w_proj = np.asarray(w_proj, dtype=np.float32)
    b_proj = np.asarray(b_proj, dtype=np.float32)

    # diagonal staircase mask [tk, q]: valid iff q >= tk
    dmask = (np.arange(P)[None, :] >= np.arange(P)[:, None]).astype(BF16)
    ident = np.eye(P, dtype=BF16)
    ident32 = np.eye(P, dtype=np.float32)

    in_maps = []
    for b in range(B):
        xT = np.ascontiguousarray(x[b].T).astype(BF16)       # [C, T]
        for hg in range(2):
            lo = hg * 512
            wqk = np.concatenate(
                [w_attn[:, lo:lo + 512], w_attn[:, 1024 + lo:1024 + lo + 512]],
                axis=1).astype(BF16)                          # [C, 1024]
            wv = w_attn[:, 2048 + lo:2048 + lo + 512].astype(BF16)
            wproj = w_proj[lo:lo + 512, :].astype(BF16)       # [512, C]
            bqk = np.stack(
                [b_attn[lo + P * m:lo + P * (m + 1)] for m in range(4)] +
                [b_attn[1024 + lo + P * m:1024 + lo + P * (m + 1)] for m in range(4)],
                axis=1).astype(np.float32)                    # [128, 8]
            bv = np.broadcast_to(b_attn[2048 + lo:2048 + lo + 512],
                                 (P, 512)).astype(np.float32)
            bp = b_proj if hg == 0 else np.zeros_like(b_proj)
            bproj = np.broadcast_to(bp, (P, C)).astype(np.float32)
            in_maps.append({
                "xT": xT, "wqk": wqk, "wv": wv, "wproj": wproj,
                "bqk": np.ascontiguousarray(bqk), "bv": np.ascontiguousarray(bv),
                "bproj": np.ascontiguousarray(bproj),
                "dmask": np.ascontiguousarray(dmask), "ident": ident,
                "ident32": ident32,
            })
    return in_maps


def kernel(x, w_attn, b_attn, w_proj, b_proj):
    global LAST_RESULT
    from concourse.bass_utils import run_bass_kernel_spmd

    if "nc" not in _CACHE:
        _CACHE["nc"] = _build_program()
    nc = _CACHE["nc"]

    in_maps = _prep_inputs(x, w_attn, b_attn, w_proj, b_proj)
    res = run_bass_kernel_spmd(nc, in_maps, core_ids=list(range(8)))
    LAST_RESULT = res

    out = np.zeros((B, T, C), dtype=np.float32)
    for b in range(B):
        out[b] = (res.results[2 * b]["out"].astype(np.float32) +
                  res.results[2 * b + 1]["out"].astype(np.float32))
    return out


# revision 15
# speedup vs baseline: 1.0697x; 1.0697x over previous
"""Causal self-attention (B=4, T=2048, C=1024, 16 heads) on 8 trn2 NeuronCores.

Sharding: core (b, hg) handles batch b (4) x head-group hg (2 groups of 8 heads).
Each core computes QKV projection for its batch restricted to its 8 heads,
flash-style causal attention, and the output projection restricted to its
heads' rows of w_proj -> a partial [T, C] output. Host sums the two partials
per batch (tensor-parallel unshard) and concatenates batches.

Key layout choices (all bf16 matmul inputs, fp32 PSUM accumulation):
  - x is fed pre-transposed per batch: xT [C, T], so Q^T/K^T [d, t] come
    straight out of the QKV matmuls (lhsT = w slice, rhs = xT).
  - Scores are computed TRANSPOSED: S^T[tk, q] = (K^T)^T-style matmul with
    lhsT = K^T chunk, rhs = Q^T chunk. Softmax exp runs on ScalarE from PSUM.
  - V carries an appended ones-column, so the A@V matmul (lhsT=[V|1], rhs=P^T,
    N=512 column-dense streams that keep the PE's HAM activity monitor at full
    clock) yields y~^T = [64 weighted-V | row-sum] x q in one pass. The [65,512]
    result is PE-transposed in 128-q blocks so the softmax denominator becomes
    per-partition -> cheap DVE reciprocal + tensor_scalar multiply.
  - y [t, ch] is PE-transposed to y^T [ch, t] for the output projection.
  - Causal masking: block-skipping, matmul column ranges narrowed to the valid
    q-range on diagonal slots, exp written only to the valid range of a
    dedicated diag buffer (whose sub-diagonal zeros are written once), and one
    [128,128] staircase mask multiplied into the true diagonal blocks.
  - Next pair's QKV projection matmuls are interleaved into the attention
    c-loop as PE filler while ScalarE works through the exps; pair 3
    interleaves the transpose/output-projection tail the same way.
"""

import numpy as np
import ml_dtypes

B, T, C, H, D = 4, 2048, 1024, 16, 64
P = 128
TC = T // P          # 16 t-chunks of 128
KC = C // P          # 8 contraction chunks of 128
NPAIR = 4            # head pairs per core (8 local heads)
SCALE = 0.125        # 1/sqrt(64)

_CACHE = {}
LAST_RESULT = None   # BassKernelResults of the most recent run (for test.py)

BF16 = ml_dtypes.bfloat16


def _build_program():
    import concourse.tile as tile
    import concourse.mybir as mybir
    from concourse import bacc

    dt = mybir.dt
    AF = mybir.ActivationFunctionType
    ALU = mybir.AluOpType

    nc = bacc.Bacc("TRN2", target_bir_lowering=False, debug=False, num_devices=8)

    # ---- DRAM I/O ----
    xT_d = nc.dram_tensor("xT", [C, T], dt.bfloat16, kind="ExternalInput").ap()
    wqk_d = nc.dram_tensor("wqk", [C, 1024], dt.bfloat16, kind="ExternalInput").ap()
    wv_d = nc.dram_tensor("wv", [C, 512], dt.bfloat16, kind="ExternalInput").ap()
    wproj_d = nc.dram_tensor("wproj", [512, C], dt.bfloat16, kind="ExternalInput").ap()
    bqk_d = nc.dram_tensor("bqk", [P, 8], dt.float32, kind="ExternalInput").ap()
    bv_d = nc.dram_tensor("bv", [P, 512], dt.float32, kind="ExternalInput").ap()
    bproj_d = nc.dram_tensor("bproj", [P, C], dt.float32, kind="ExternalInput").ap()
    dmask_d = nc.dram_tensor("dmask", [P, P], dt.bfloat16, kind="ExternalInput").ap()
    ident_d = nc.dram_tensor("ident", [P, P], dt.bfloat16, kind="ExternalInput").ap()
    ident32_d = nc.dram_tensor("ident32", [P, P], dt.float32, kind="ExternalInput").ap()
    out_d = nc.dram_tensor("out", [T, C], dt.bfloat16, kind="ExternalOutput").ap()

    with tile.TileContext(nc) as tc:
        with (
            tc.tile_pool(name="const", bufs=1) as cp,
            tc.tile_pool(name="outp", bufs=3) as op_pool,
            tc.tile_pool(name="small", bufs=4) as sp,
            tc.tile_pool(name="ytmp", bufs=2) as yt_pool,
            tc.tile_pool(name="psqk", bufs=2, space="PSUM") as psqk_pool,
            tc.tile_pool(name="psmm", bufs=2, space="PSUM") as psmm_pool,
            tc.tile_pool(name="pstr", bufs=2, space="PSUM") as pstr_pool,
        ):
            # ---- static SBUF tensors ----
            xT_s = cp.tile([P, KC, T], dt.bfloat16, name="xT_s")
            wqk_s = cp.tile([P, KC, 1024], dt.bfloat16, name="wqk_s")
            wv_s = cp.tile([P, KC, 512], dt.bfloat16, name="wv_s")
            wproj_s = cp.tile([P, 4, C], dt.bfloat16, name="wproj_s")
            bqk_s = cp.tile([P, 8], dt.float32, name="bqk_s")
            bv_s = cp.tile([P, 512], dt.float32, name="bv_s")
            bproj_s = cp.tile([P, C], dt.float32, name="bproj_s")
            dmask_s = cp.tile([P, P], dt.bfloat16, name="dmask_s")
            ident_s = cp.tile([P, P], dt.bfloat16, name="ident_s")
            ident32_s = cp.tile([P, P], dt.float32, name="ident32_s")
            qt_s = cp.tile([P, NPAIR, T], dt.bfloat16, name="qt_s")  # later reused as y^T
            kt_s = cp.tile([P, NPAIR, T], dt.bfloat16, name="kt_s")
            v_s = cp.tile([P, TC, 8, 66], dt.bfloat16, name="v_s")   # [t, tc, head, V|1]
            y_s = cp.tile([P, TC, 8, D], dt.bfloat16, name="y_s")    # y natural [q, head, d]
            pt_s = cp.tile([P, 2, 6, 1024], dt.bfloat16, name="pt_s")   # exp(S^T), off-diag groups
            ptd_s = cp.tile([P, 2, 2, 1024], dt.bfloat16, name="ptd_s")  # diag groups (zeros persist)

            # ---- input DMAs (weights for the first compute first) ----
            xT_src = xT_d.rearrange("(o p) t -> p o t", p=P)
            wv_src = wv_d.rearrange("(o p) m -> p o m", p=P)
            wqk_src = wqk_d.rearrange("(o p) m -> p o m", p=P)
            nc.sync.dma_start(wv_s[:, 0:4, :], wv_src[:, 0:4, :])
            nc.sync.dma_start(xT_s[:, :, 0:256], xT_src[:, :, 0:256])
            nc.sync.dma_start(wv_s[:, 4:8, :], wv_src[:, 4:8, :])
            nc.sync.dma_start(bv_s[:], bv_d)
            nc.sync.dma_start(xT_s[:, :, 256:512], xT_src[:, :, 256:512])
            nc.sync.dma_start(wqk_s[:, 0:4, :], wqk_src[:, 0:4, :])
            nc.sync.dma_start(xT_s[:, :, 512:768], xT_src[:, :, 512:768])
            nc.sync.dma_start(wqk_s[:, 4:8, :], wqk_src[:, 4:8, :])
            nc.sync.dma_start(bqk_s[:], bqk_d)
            for q8 in range(3, 8):
                nc.sync.dma_start(xT_s[:, :, 256 * q8:256 * (q8 + 1)],
                                  xT_src[:, :, 256 * q8:256 * (q8 + 1)])
            nc.sync.dma_start(dmask_s[:], dmask_d)
            nc.sync.dma_start(ident_s[:], ident_d)
            nc.sync.dma_start(ident32_s[:], ident32_d)
            nc.sync.dma_start(wproj_s[:], wproj_d.rearrange("(o p) m -> p o m", p=P))
            nc.sync.dma_start(bproj_s[:], bproj_d)

            # ones column of V~; zero the diag P^T buffer once (sub-diagonal
            # regions are never written by the partial exps, so zeros persist)
            nc.vector.memset(v_s[:, :, :, 64:65], 1.0)
            nc.vector.memset(ptd_s[:], 0.0)

            # ---- V projection: v[t, ch] for all 8 heads (512 cols) ----
            for tcx in range(TC):
                psv = psmm_pool.tile([P, 512], dt.float32, name="psv", tag="mm")
                for k in range(KC):
                    nc.tensor.matmul(psv[:, :],
                                     xT_s[:, k, P * tcx:P * (tcx + 1)],
                                     wv_s[:, k, :],
                                     start=(k == 0), stop=(k == KC - 1))
                nc.vector.tensor_add(
                    out=v_s[:, tcx, :, 0:64],
                    in0=psv[:, :].rearrange("a (h d) -> a h d", h=8),
                    in1=bv_s[:, :].rearrange("a (h d) -> a h d", h=8),
                )

            # ---- helper emitters ----
            def qkproj_chunk(m, t4):
                """One [128 out-ch, 512 t] tile of the Q^T/K^T projection."""
                dst = qt_s if m < 4 else kt_s
                psq = psmm_pool.tile([P, 512], dt.float32, name="psq", tag="mm")
                for k in range(KC):
                    nc.tensor.matmul(psq[:, :],
                                     wqk_s[:, k, P * m:P * (m + 1)],
                                     xT_s[:, k, 512 * t4:512 * (t4 + 1)],
                                     start=(k == 0), stop=(k == KC - 1))
                nc.vector.tensor_scalar(
                    out=dst[:, m % 4, 512 * t4:512 * (t4 + 1)],
                    in0=psq[:, :], scalar1=bqk_s[:, m:m + 1], scalar2=None,
                    op0=ALU.add)

            def transpose_chunk(tcx, cc):
                pst = pstr_pool.tile([P, P], dt.bfloat16, name="pst", tag="tr")
                nc.tensor.transpose(pst[:, :], y_s[:, tcx, 2 * cc:2 * cc + 2, :],
                                    ident_s[:, :])
                nc.scalar.copy(out=qt_s[:, cc, P * tcx:P * (tcx + 1)],
                               in_=pst[:, :])

            def proj_chunk(tcx):
                for co in range(2):
                    psp = psmm_pool.tile([P, 512], dt.float32, name="psp", tag="mm")
                    for cc in range(4):
                        nc.tensor.matmul(psp[:, :],
                                         qt_s[:, cc, P * tcx:P * (tcx + 1)],
                                         wproj_s[:, cc, 512 * co:512 * (co + 1)],
                                         start=(cc == 0), stop=(cc == 3))
                    ot = op_pool.tile([P, 512], dt.bfloat16, name="ot", tag="ot")
                    nc.vector.tensor_add(out=ot[:, :], in0=psp[:, :],
                                         in1=bproj_s[:, 512 * co:512 * (co + 1)])
                    nc.sync.dma_start(
                        out_d[P * tcx:P * (tcx + 1), 512 * co:512 * (co + 1)], ot[:, :])

            # ---- pair 0 projection upfront; later pairs interleave ----
            for m in (0, 4):
                for t4 in range(4):
                    qkproj_chunk(m, t4)

            for pair in range(NPAIR):
                nxt = ([(m, t4) for m in (pair + 1, 5 + pair) for t4 in range(4)]
                       if pair < NPAIR - 1 else [])
                for ci, c in enumerate(range(4)):   # q chunk of 512
                    ngroups = 2 * c + 2     # tk-chunk pairs: j = 2g, 2g+1
                    for g in range(ngroups):
                        diag = g >= 2 * c
                        # both heads' score tiles; MMs interleaved h0/h1 so
                        # row-tiled (K=64) pairs overlap in the PE array
                        psS = [psqk_pool.tile([P, 1024], dt.float32, name="psS",
                                              tag="psqk") for _ in (0, 1)]
                        for s in (0, 1):
                            j = 2 * g + s
                            q0 = P * (j - 4 * c) if diag else 0  # skip masked cols
                            for hh in (0, 1):
                                base = 64 * hh
                                nc.tensor.matmul(
                                    psS[hh][:, 512 * s + q0:512 * (s + 1)],
                                    kt_s[base:base + 64, pair, P * j:P * (j + 1)],
                                    qt_s[base:base + 64, pair,
                                         512 * c + q0:512 * (c + 1)],
                                    start=True, stop=True)
                        # exp( S^T * scale ), fp32 psum -> bf16 sbuf
                        for hh in (0, 1):
                            if not diag:
                                nc.scalar.activation(pt_s[:, hh, g, :], psS[hh][:, :],
                                                     AF.Exp, scale=SCALE)
                            else:
                                gd = g - 2 * c
                                for s in (0, 1):
                                    r = 2 * gd + s
                                    q0 = 512 * s + P * r
                                    nc.scalar.activation(
                                        ptd_s[:, hh, gd, q0:512 * (s + 1)],
                                        psS[hh][:, q0:512 * (s + 1)],
                                        AF.Exp, scale=SCALE)
                                    # staircase mask on the true diagonal block
                                    nc.vector.tensor_tensor(
                                        out=ptd_s[:, hh, gd, q0:q0 + P],
                                        in0=ptd_s[:, hh, gd, q0:q0 + P],
                                        in1=dmask_s[:, :], op=ALU.mult)

                    # PE filler while ScalarE works through the exps:
                    # next pair's Q^T/K^T projection, 2 chunks per c
                    for (m, t4) in nxt[2 * ci:2 * ci + 2]:
                        qkproj_chunk(m, t4)

                    # [V | 1]^T @ P^T per head: y~^T [65, 512] column-dense
                    # (keeps the PE MAC-active so HAM stays at full clock),
                    # then PE-transpose 128-col blocks to get the softmax
                    # denominator per-partition for the division
                    for hh in (0, 1):
                        h = 2 * pair + hh
                        nj = 4 * c + 4
                        psyt = psmm_pool.tile([P, 512], dt.float32, name="psyt",
                                              tag="mm")
                        for j in range(nj):
                            g, s = j // 2, j % 2
                            if g < 2 * c:
                                rhs = pt_s[:, hh, g, 512 * s:512 * (s + 1)]
                                out = psyt[0:65, :]
                            else:
                                # diagonal slot: only columns q >= 128r are live
                                r = j - 4 * c
                                q0 = P * r
                                rhs = ptd_s[:, hh, g - 2 * c,
                                            512 * s + q0:512 * (s + 1)]
                                out = psyt[0:65, q0:]
                            nc.tensor.matmul(
                                out, v_s[:, j, h, 0:65], rhs,
                                start=(j == 0), stop=(j == nj - 1))
                        ytmp = yt_pool.tile([P, 512], dt.float32, name="ytmp",
                                            tag="ytmp")
                        nc.vector.tensor_copy(out=ytmp[0:65, :], in_=psyt[0:65, :])
                        for qi_loc in range(4):
                            qi = 4 * c + qi_loc
                            ptr = pstr_pool.tile([P, P], dt.float32, name="ptr",
                                                 tag="tr")
                            nc.tensor.transpose(
                                ptr[:, 0:65],
                                ytmp[0:65, P * qi_loc:P * (qi_loc + 1)],
                                ident32_s[0:65, 0:65])
                            linv = sp.tile([P, 1], dt.float32, name="linv", tag="linv")
                            nc.vector.reciprocal(linv[:, :], ptr[:, 64:65])
                            nc.vector.tensor_scalar(
                                out=y_s[:, qi, h, :], in0=ptr[:, 0:64],
                                scalar1=linv[:, :], scalar2=None, op0=ALU.mult)

                    if pair == NPAIR - 1:
                        # tail pipelined into pair 3: transpose y -> y^T
                        # (reusing qt_s) and run the output projection for the
                        # t-chunks whose y rows just completed
                        for qi_loc in range(4):
                            tcx = 4 * c + qi_loc
                            for cc in range(4):
                                transpose_chunk(tcx, cc)
                        for qi_loc in range(4):
                            proj_chunk(4 * c + qi_loc)

    nc.compile()
    return nc


def _prep_inputs(x, w_attn, b_attn, w_proj, b_proj):
    """Host-side shard prep: per-core input dicts (core ci = b*2 + hg)."""
    x = np.asarray(x, dtype=np.float32)
    w_attn = np.asarray(w_attn, dtype=np.float32)
    b_attn = np.asarray(b_attn, dtype=np.float32)
    w_proj = np.asarray(w_proj, dtype=np.float32)
    b_proj = np.asarray(b_proj, dtype=np.float32)

    # diagonal staircase mask [tk, q]: valid iff q >= tk
    dmask = (np.arange(P)[None, :] >= np.arange(P)[:, None]).astype(BF16)
    ident = np.eye(P, dtype=BF16)
    ident32 = np.eye(P, dtype=np.float32)

    in_maps = []
    for b in range(B):
        xT = np.ascontiguousarray(x[b].T).astype(BF16)       # [C, T]
        for hg in range(2):
            lo = hg * 512
            wqk = np.concatenate(
                [w_attn[:, lo:lo + 512], w_attn[:, 1024 + lo:1024 + lo + 512]],
                axis=1).astype(BF16)                          # [C, 1024]
            wv = w_attn[:, 2048 + lo:2048 + lo + 512].astype(BF16)
            wproj = w_proj[lo:lo + 512, :].astype(BF16)       # [512, C]
            bqk = np.stack(
                [b_attn[lo + P * m:lo + P * (m + 1)] for m in range(4)] +
                [b_attn[1024 + lo + P * m:1024 + lo + P * (m + 1)] for m in range(4)],
                axis=1).astype(np.float32)                    # [128, 8]
            bv = np.broadcast_to(b_attn[2048 + lo:2048 + lo + 512],
                                 (P, 512)).astype(np.float32)
            bp = b_proj if hg == 0 else np.zeros_like(b_proj)
            bproj = np.broadcast_to(bp, (P, C)).astype(np.float32)
            in_maps.append({
                "xT": xT, "wqk": wqk, "wv": wv, "wproj": wproj,
                "bqk": np.ascontiguousarray(bqk), "bv": np.ascontiguousarray(bv),
                "bproj": np.ascontiguousarray(bproj),
                "dmask": np.ascontiguousarray(dmask), "ident": ident,
                "ident32": ident32,
            })
    return in_maps


def kernel(x, w_attn, b_attn, w_proj, b_proj):
    global LAST_RESULT
    from concourse.bass_utils import run_bass_kernel_spmd

    if "nc" not in _CACHE:
        _CACHE["nc"] = _build_program()
    nc = _CACHE["nc"]

    in_maps = _prep_inputs(x, w_attn, b_attn, w_proj, b_proj)
    res = run_bass_kernel_spmd(nc, in_maps, core_ids=list(range(8)))
    LAST_RESULT = res

    out = np.zeros((B, T, C), dtype=np.float32)
    for b in range(B):
        out[b] = (res.results[2 * b]["out"].astype(np.float32) +
                  res.results[2 * b + 1]["out"].astype(np.float32))
    return out


# revision 16
# speedup vs baseline: 1.1140x; 1.0414x over previous
"""Causal self-attention (B=4, T=2048, C=1024, 16 heads) on 8 trn2 NeuronCores.

Sharding: core (b, hg) handles batch b (4) x head-group hg (2 groups of 8 heads).
Each core computes QKV projection for its batch restricted to its 8 heads,
flash-style causal attention, and the output projection restricted to its
heads' rows of w_proj -> a partial [T, C] output. Host sums the two partials
per batch (tensor-parallel unshard) and concatenates batches.

Key layout choices (all bf16 matmul inputs, fp32 PSUM accumulation):
  - x is fed pre-transposed per batch: xT [C, T], so Q^T/K^T [d, t] come
    straight out of the QKV matmuls (lhsT = w slice, rhs = xT).
  - Scores are computed TRANSPOSED: S^T[tk, q] = (K^T)^T-style matmul with
    lhsT = K^T chunk, rhs = Q^T chunk. Softmax exp runs on ScalarE from PSUM.
  - V carries an appended ones-column, so the A@V matmul (lhsT=[V|1], rhs=P^T,
    N=512 column-dense streams that keep the PE's HAM activity monitor at full
    clock) yields y~^T = [64 weighted-V | row-sum] x q in one pass. The [65,512]
    result is PE-transposed in 128-q blocks so the softmax denominator becomes
    per-partition -> cheap DVE reciprocal + tensor_scalar multiply.
  - y [t, ch] is PE-transposed to y^T [ch, t] for the output projection.
  - Causal masking: block-skipping, matmul column ranges narrowed to the valid
    q-range on diagonal slots, exp written only to the valid range of a
    dedicated diag buffer (whose sub-diagonal zeros are written once), and one
    [128,128] staircase mask multiplied into the true diagonal blocks.
  - Next pair's QKV projection matmuls are interleaved into the attention
    c-loop as PE filler while ScalarE works through the exps; pair 3
    interleaves the transpose/output-projection tail the same way.
"""

import numpy as np
import ml_dtypes

B, T, C, H, D = 4, 2048, 1024, 16, 64
P = 128
TC = T // P          # 16 t-chunks of 128
KC = C // P          # 8 contraction chunks of 128
NPAIR = 4            # head pairs per core (8 local heads)
SCALE = 0.125        # 1/sqrt(64)

_CACHE = {}
LAST_RESULT = None   # BassKernelResults of the most recent run (for test.py)

BF16 = ml_dtypes.bfloat16


def _build_program():
    import concourse.tile as tile
    import concourse.mybir as mybir
    from concourse import bacc

    dt = mybir.dt
    AF = mybir.ActivationFunctionType
    ALU = mybir.AluOpType

    nc = bacc.Bacc("TRN2", target_bir_lowering=False, debug=False, num_devices=8)

    # ---- DRAM I/O ----
    xT_d = nc.dram_tensor("xT", [C, T], dt.bfloat16, kind="ExternalInput").ap()
    wqk_d = nc.dram_tensor("wqk", [C, 1024], dt.bfloat16, kind="ExternalInput").ap()
    wv_d = nc.dram_tensor("wv", [C, 512], dt.bfloat16, kind="ExternalInput").ap()
    wproj_d = nc.dram_tensor("wproj", [512, C], dt.bfloat16, kind="ExternalInput").ap()
    bqk_d = nc.dram_tensor("bqk", [P, 8], dt.float32, kind="ExternalInput").ap()
    bv_d = nc.dram_tensor("bv", [P, 512], dt.float32, kind="ExternalInput").ap()
    bproj_d = nc.dram_tensor("bproj", [P, C], dt.float32, kind="ExternalInput").ap()
    dmask_d = nc.dram_tensor("dmask", [P, P], dt.bfloat16, kind="ExternalInput").ap()
    ident_d = nc.dram_tensor("ident", [P, P], dt.bfloat16, kind="ExternalInput").ap()
    ident32_d = nc.dram_tensor("ident32", [P, P], dt.float32, kind="ExternalInput").ap()
    out_d = nc.dram_tensor("out", [T, C], dt.bfloat16, kind="ExternalOutput").ap()

    with tile.TileContext(nc) as tc:
        with (
            tc.tile_pool(name="const", bufs=1) as cp,
            tc.tile_pool(name="outp", bufs=3) as op_pool,
            tc.tile_pool(name="small", bufs=4) as sp,
            tc.tile_pool(name="ytmp", bufs=2) as yt_pool,
            tc.tile_pool(name="psqk", bufs=2, space="PSUM") as psqk_pool,
            tc.tile_pool(name="psmm", bufs=2, space="PSUM") as psmm_pool,
            tc.tile_pool(name="pstr", bufs=2, space="PSUM") as pstr_pool,
        ):
            # ---- static SBUF tensors ----
            xT_s = cp.tile([P, KC, T], dt.bfloat16, name="xT_s")
            wqk_s = cp.tile([P, KC, 1024], dt.bfloat16, name="wqk_s")
            wv_s = cp.tile([P, KC, 512], dt.bfloat16, name="wv_s")
            wproj_s = cp.tile([P, 4, C], dt.bfloat16, name="wproj_s")
            bqk_s = cp.tile([P, 8], dt.float32, name="bqk_s")
            bv_s = cp.tile([P, 512], dt.float32, name="bv_s")
            bproj_s = cp.tile([P, C], dt.float32, name="bproj_s")
            dmask_s = cp.tile([P, P], dt.bfloat16, name="dmask_s")
            ident_s = cp.tile([P, P], dt.bfloat16, name="ident_s")
            ident32_s = cp.tile([P, P], dt.float32, name="ident32_s")
            qt_s = cp.tile([P, NPAIR, T], dt.bfloat16, name="qt_s")  # later reused as y^T
            kt_s = cp.tile([P, NPAIR, T], dt.bfloat16, name="kt_s")
            v_s = cp.tile([P, TC, 8, 66], dt.bfloat16, name="v_s")   # [t, tc, head, V|1]
            y_s = cp.tile([P, TC, 8, D], dt.bfloat16, name="y_s")    # y natural [q, head, d]
            pt_s = cp.tile([P, 12, 2, 512], dt.bfloat16, name="pt_s")   # exp(S^T) off-diag [slot,hh,q]
            ptd_s = cp.tile([P, 4, 2, 512], dt.bfloat16, name="ptd_s")  # diag slots (zeros persist)

            # ---- input DMAs (weights for the first compute first) ----
            xT_src = xT_d.rearrange("(o p) t -> p o t", p=P)
            wv_src = wv_d.rearrange("(o p) m -> p o m", p=P)
            wqk_src = wqk_d.rearrange("(o p) m -> p o m", p=P)
            nc.sync.dma_start(wv_s[:, 0:4, :], wv_src[:, 0:4, :])
            nc.sync.dma_start(xT_s[:, :, 0:256], xT_src[:, :, 0:256])
            nc.sync.dma_start(wv_s[:, 4:8, :], wv_src[:, 4:8, :])
            nc.sync.dma_start(bv_s[:], bv_d)
            nc.sync.dma_start(xT_s[:, :, 256:512], xT_src[:, :, 256:512])
            nc.sync.dma_start(wqk_s[:, 0:4, :], wqk_src[:, 0:4, :])
            nc.sync.dma_start(xT_s[:, :, 512:768], xT_src[:, :, 512:768])
            nc.sync.dma_start(wqk_s[:, 4:8, :], wqk_src[:, 4:8, :])
            nc.sync.dma_start(bqk_s[:], bqk_d)
            for q8 in range(3, 8):
                nc.sync.dma_start(xT_s[:, :, 256 * q8:256 * (q8 + 1)],
                                  xT_src[:, :, 256 * q8:256 * (q8 + 1)])
            nc.sync.dma_start(dmask_s[:], dmask_d)
            nc.sync.dma_start(ident_s[:], ident_d)
            nc.sync.dma_start(ident32_s[:], ident32_d)
            nc.sync.dma_start(wproj_s[:], wproj_d.rearrange("(o p) m -> p o m", p=P))
            nc.sync.dma_start(bproj_s[:], bproj_d)

            # ones column of V~; zero the diag P^T buffer once (sub-diagonal
            # regions are never written by the partial exps, so zeros persist)
            nc.vector.memset(v_s[:, :, :, 64:65], 1.0)
            nc.vector.memset(ptd_s[:], 0.0)

            # ---- V projection: v[t, ch] for all 8 heads (512 cols) ----
            for tcx in range(TC):
                psv = psmm_pool.tile([P, 512], dt.float32, name="psv", tag="mm")
                for k in range(KC):
                    nc.tensor.matmul(psv[:, :],
                                     xT_s[:, k, P * tcx:P * (tcx + 1)],
                                     wv_s[:, k, :],
                                     start=(k == 0), stop=(k == KC - 1))
                nc.vector.tensor_add(
                    out=v_s[:, tcx, :, 0:64],
                    in0=psv[:, :].rearrange("a (h d) -> a h d", h=8),
                    in1=bv_s[:, :].rearrange("a (h d) -> a h d", h=8),
                )

            # ---- helper emitters ----
            def qkproj_chunk(m, t4):
                """One [128 out-ch, 512 t] tile of the Q^T/K^T projection."""
                dst = qt_s if m < 4 else kt_s
                psq = psmm_pool.tile([P, 512], dt.float32, name="psq", tag="mm")
                for k in range(KC):
                    nc.tensor.matmul(psq[:, :],
                                     wqk_s[:, k, P * m:P * (m + 1)],
                                     xT_s[:, k, 512 * t4:512 * (t4 + 1)],
                                     start=(k == 0), stop=(k == KC - 1))
                nc.vector.tensor_scalar(
                    out=dst[:, m % 4, 512 * t4:512 * (t4 + 1)],
                    in0=psq[:, :], scalar1=bqk_s[:, m:m + 1], scalar2=None,
                    op0=ALU.add)

            def transpose_chunk(tcx, cc):
                pst = pstr_pool.tile([P, P], dt.bfloat16, name="pst", tag="tr")
                nc.tensor.transpose(pst[:, :], y_s[:, tcx, 2 * cc:2 * cc + 2, :],
                                    ident_s[:, :])
                nc.scalar.copy(out=qt_s[:, cc, P * tcx:P * (tcx + 1)],
                               in_=pst[:, :])

            def proj_chunk(tcx):
                for co in range(2):
                    psp = psmm_pool.tile([P, 512], dt.float32, name="psp", tag="mm")
                    for cc in range(4):
                        nc.tensor.matmul(psp[:, :],
                                         qt_s[:, cc, P * tcx:P * (tcx + 1)],
                                         wproj_s[:, cc, 512 * co:512 * (co + 1)],
                                         start=(cc == 0), stop=(cc == 3))
                    ot = op_pool.tile([P, 512], dt.bfloat16, name="ot", tag="ot")
                    nc.vector.tensor_add(out=ot[:, :], in0=psp[:, :],
                                         in1=bproj_s[:, 512 * co:512 * (co + 1)])
                    nc.sync.dma_start(
                        out_d[P * tcx:P * (tcx + 1), 512 * co:512 * (co + 1)], ot[:, :])

            # ---- pair 0 projection upfront; later pairs interleave ----
            for m in (0, 4):
                for t4 in range(4):
                    qkproj_chunk(m, t4)

            for pair in range(NPAIR):
                nxt = ([(m, t4) for m in (pair + 1, 5 + pair) for t4 in range(4)]
                       if pair < NPAIR - 1 else [])
                for ci, c in enumerate(range(4)):   # q chunk of 512
                    for j in range(4 * c + 4):          # tk chunk (slot)
                        r = j - 4 * c                   # >= 0 on diagonal slots
                        q0 = P * r if r >= 0 else 0     # skip masked cols
                        psS = psqk_pool.tile([P, 1024], dt.float32, name="psS",
                                             tag="psqk")
                        for hh in (0, 1):
                            base = 64 * hh
                            nc.tensor.matmul(
                                psS[:, 512 * hh + q0:512 * (hh + 1)],
                                kt_s[base:base + 64, pair, P * j:P * (j + 1)],
                                qt_s[base:base + 64, pair,
                                     512 * c + q0:512 * (c + 1)],
                                start=True, stop=True)
                        # exp( S^T * scale ), fp32 psum -> bf16 sbuf
                        if r < 0:
                            nc.scalar.activation(pt_s[:, j, :, :], psS[:, :],
                                                 AF.Exp, scale=SCALE)
                        else:
                            for hh in (0, 1):
                                nc.scalar.activation(
                                    ptd_s[:, r, hh, q0:],
                                    psS[:, 512 * hh + q0:512 * (hh + 1)],
                                    AF.Exp, scale=SCALE)
                                # staircase mask on the true diagonal block
                                nc.vector.tensor_tensor(
                                    out=ptd_s[:, r, hh, q0:q0 + P],
                                    in0=ptd_s[:, r, hh, q0:q0 + P],
                                    in1=dmask_s[:, :], op=ALU.mult)

                    # PE filler while ScalarE works through the exps:
                    # next pair's Q^T/K^T projection, 2 chunks per c
                    for (m, t4) in nxt[2 * ci:2 * ci + 2]:
                        qkproj_chunk(m, t4)

                    # [V | 1]^T @ P^T per head: y~^T [65, 512] column-dense
                    # (keeps the PE MAC-active so HAM stays at full clock),
                    # then PE-transpose 128-col blocks to get the softmax
                    # denominator per-partition for the division
                    for hh in (0, 1):
                        h = 2 * pair + hh
                        nj = 4 * c + 4
                        psyt = psmm_pool.tile([P, 512], dt.float32, name="psyt",
                                              tag="mm")
                        for j in range(nj):
                            r = j - 4 * c
                            if r < 0:
                                rhs = pt_s[:, j, hh, :]
                                out = psyt[0:65, :]
                            else:
                                # diagonal slot: only columns q >= 128r are live
                                rhs = ptd_s[:, r, hh, P * r:]
                                out = psyt[0:65, P * r:]
                            nc.tensor.matmul(
                                out, v_s[:, j, h, 0:65], rhs,
                                start=(j == 0), stop=(j == nj - 1))
                        ytmp = yt_pool.tile([P, 512], dt.float32, name="ytmp",
                                            tag="ytmp")
                        nc.vector.tensor_copy(out=ytmp[0:65, :], in_=psyt[0:65, :])
                        for qi_loc in range(4):
                            qi = 4 * c + qi_loc
                            ptr = pstr_pool.tile([P, P], dt.float32, name="ptr",
                                                 tag="tr")
                            nc.tensor.transpose(
                                ptr[:, 0:65],
                                ytmp[0:65, P * qi_loc:P * (qi_loc + 1)],
                                ident32_s[0:65, 0:65])
                            linv = sp.tile([P, 1], dt.float32, name="linv", tag="linv")
                            nc.vector.reciprocal(linv[:, :], ptr[:, 64:65])
                            nc.vector.tensor_scalar(
                                out=y_s[:, qi, h, :], in0=ptr[:, 0:64],
                                scalar1=linv[:, :], scalar2=None, op0=ALU.mult)

                    if pair == NPAIR - 1:
                        # tail pipelined into pair 3: transpose y -> y^T
                        # (reusing qt_s) and run the output projection for the
                        # t-chunks whose y rows just completed
                        for qi_loc in range(4):
                            tcx = 4 * c + qi_loc
                            for cc in range(4):
                                transpose_chunk(tcx, cc)
                        for qi_loc in range(4):
                            proj_chunk(4 * c + qi_loc)

    nc.compile()
    return nc


def _prep_inputs(x, w_attn, b_attn, w_proj, b_proj):
    """Host-side shard prep: per-core input dicts (core ci = b*2 + hg)."""
    x = np.asarray(x, dtype=np.float32)
    w_attn = np.asarray(w_attn, dtype=np.float32)
    b_attn = np.asarray(b_attn, dtype=np.float32)
    w_proj = np.asarray(w_proj, dtype=np.float32)
    b_proj = np.asarray(b_proj, dtype=np.float32)

    # diagonal staircase mask [tk, q]: valid iff q >= tk
    dmask = (np.arange(P)[None, :] >= np.arange(P)[:, None]).astype(BF16)
    ident = np.eye(P, dtype=BF16)
    ident32 = np.eye(P, dtype=np.float32)

    in_maps = []
    for b in range(B):
        xT = np.ascontiguousarray(x[b].T).astype(BF16)       # [C, T]
        for hg in range(2):
            lo = hg * 512
            wqk = np.concatenate(
                [w_attn[:, lo:lo + 512], w_attn[:, 1024 + lo:1024 + lo + 512]],
                axis=1).astype(BF16)                          # [C, 1024]
            wv = w_attn[:, 2048 + lo:2048 + lo + 512].astype(BF16)
            wproj = w_proj[lo:lo + 512, :].astype(BF16)       # [512, C]
            bqk = np.stack(
                [b_attn[lo + P * m:lo + P * (m + 1)] for m in range(4)] +
                [b_attn[1024 + lo + P * m:1024 + lo + P * (m + 1)] for m in range(4)],
                axis=1).astype(np.float32)                    # [128, 8]
            bv = np.broadcast_to(b_attn[2048 + lo:2048 + lo + 512],
                                 (P, 512)).astype(np.float32)
            bp = b_proj if hg == 0 else np.zeros_like(b_proj)
            bproj = np.broadcast_to(bp, (P, C)).astype(np.float32)
            in_maps.append({
                "xT": xT, "wqk": wqk, "wv": wv, "wproj": wproj,
                "bqk": np.ascontiguousarray(bqk), "bv": np.ascontiguousarray(bv),
                "bproj": np.ascontiguousarray(bproj),
                "dmask": np.ascontiguousarray(dmask), "ident": ident,
                "ident32": ident32,
            })
    return in_maps


def kernel(x, w_attn, b_attn, w_proj, b_proj):
    global LAST_RESULT
    from concourse.bass_utils import run_bass_kernel_spmd

    if "nc" not in _CACHE:
        _CACHE["nc"] = _build_program()
    nc = _CACHE["nc"]

    in_maps = _prep_inputs(x, w_attn, b_attn, w_proj, b_proj)
    res = run_bass_kernel_spmd(nc, in_maps, core_ids=list(range(8)))
    LAST_RESULT = res

    out = np.zeros((B, T, C), dtype=np.float32)
    for b in range(B):
        out[b] = (res.results[2 * b]["out"].astype(np.float32) +
                  res.results[2 * b + 1]["out"].astype(np.float32))
    return out
